# revision 34
# baseline (speedup 1.0000x reference)
"""Trainium2 Bass kernel for 12-head causal multi-head attention (bf16).

Problem: B=8, T=1024, C=768, H=12, HS=64, fp32 in/out.
Sharding: data-parallel over batch — core b computes batch element b.

All SBUF-resident data is bf16: matmuls run at the same 1 cycle/row rate
as fp32r (and with no narrow-tile penalty), while DMA bytes, SBUF
footprint, and DVE element cost all halve. Max rel err ~4e-3 (budget
2e-2). The output y is stored bf16 and upcast on host.

The measured per-exec time through the PJRT/axon path is dominated by
per-launch dispatch overhead that scales with the ARGUMENT COUNT
(~35us/buffer), not with device time (~124us, TimelineSim). Hence the
single merged ExternalInput (xw = xT|wqk|wv|wp|bp, all bf16) and
enable_partition_id=False: 2 exec args instead of 7, worth ~145us/exec
in a paired A/B. An fp8e4m3 DoubleRow variant of the QK^T matmul
(FP8_SCORES below) is numerically fine and ~3.5us faster in the cost
model, but measured ~30us SLOWER on hardware — default off.

Per-core algorithm (everything PE-friendly, no on-chip transposes):
  - host passes ONE merged bf16 buffer per core: x[b] TRANSPOSED
    (xT [C, T]) followed by the repacked weights and bias — a single
    ExternalInput keeps per-exec dispatch overhead minimal
  - qT/kT computed in [head*64, T] layout (head pairs packed into 128
    partitions), v in token-major [T, head*65] layout with a ones column
    appended per head (the ones column makes the PV matmul emit softmax
    row-sums for free)
  - scores are computed TRANSPOSED (s on partitions, t on free dim), one
    matmul + one Exp + (diagonal tiles only) one DVE triangle-mask
    multiply PER HEAD, so the first PV of a tile can start after half
    the exp latency
  - the whole attention runs as ONE flat software pipeline over stages
    (pair, qc, k-tile): scores/exp stay a constant 2 tiles ahead of PV,
    across qc and pair boundaries, so the in-order PE queue never drains
    at a boundary; qkT matmuls of pair p+1, and deferred normalization,
    are dripped into the PV stream as fillers
  - PV produces outT [d, T] per head pair, which is exactly the lhsT
    layout the output projection needs; per-query normalization takes a
    reciprocal of the PSUM row-sum row, broadcasts BOTH head rows across
    128 partitions with a single selector matmul, and multiplies
  - DMAs are ordered so the serial ~358GB/s HBM engine delivers exactly
    what the PE needs first (wqk chunk 0 + first half of xT chunk 0);
    proj weights load mid-kernel
"""

import os
import numpy as np
import ml_dtypes

B, T, C = 8, 1024, 768
H, HS = 12, 64
NPAIR = 6  # head pairs (2 heads of 64 -> 128 partitions)
NCK = 6    # contraction chunks of 128 over C
NT = 8     # token tiles of 128

# fp8 DoubleRow scores (q/k quantized to e4m3 on-chip, QK^T at 2 rows/
# cycle): correct on HW (rel err 1.40e-2 < 2e-2) and ~3.5us faster in
# the cost model, but measured ~30us SLOWER on real silicon in a paired
# A/B (extra SWDGE remap DMAs + DoubleRow overhead). Kept for reference,
# default off.
FP8_SCORES = False

# single merged input buffer (bf16 elements): xT | wqk | wv | wp | bp.
# One ExternalInput instead of five: per-exec dispatch cost through the
# PJRT/axon path scales with argument count (~35us/arg), so the merged
# layout cuts real per-launch overhead.
OXT = 0                      # xT   [C, T]           (c*T + t)
OWQK = OXT + C * T           # wqk  [6, 128, 6, 256]
OWV = OWQK + NPAIR * 128 * NCK * 256   # wv [C, C]
OWP = OWV + C * C            # wp   [6, 128, C]
OBP = OWP + NPAIR * 128 * C  # bp   [C] (stored bf16)
NXW = OBP + C

LAST_EXEC_NS = None
LAST_RESULTS = None

_cached_nc = None


def _build_nc():
    import concourse.bass as bass
    import concourse.mybir as mybir
    import concourse.tile as tile
    from concourse import bacc
    from concourse.masks import make_upper_triangular

    f32 = mybir.dt.float32
    bf16 = mybir.dt.bfloat16
    f8 = mybir.dt.float8e4
    DR = mybir.MatmulPerfMode.DoubleRow
    AF = mybir.ActivationFunctionType

    nc = bacc.Bacc("TRN2", target_bir_lowering=False, debug=False,
                   num_devices=8, enable_partition_id=False)

    xw_d = nc.dram_tensor("xw", [NXW], bf16, kind="ExternalInput")
    y_d = nc.dram_tensor("y", [T, C], bf16, kind="ExternalOutput")

    def xw_ap(offset, ap):
        return bass.AP(tensor=xw_d, offset=offset, ap=ap)

    with tile.TileContext(nc) as tc:
        with (
            tc.tile_pool(name="const", bufs=1) as const,
            tc.tile_pool(name="work", bufs=2) as work,
            tc.tile_pool(name="ppool", bufs=4) as ppool,
            tc.tile_pool(name="opool", bufs=1) as opool,
            tc.tile_pool(name="ps1", bufs=2, space="PSUM") as ps1,
        ):
            # ---------- resident inputs / constants ----------
            # DMA ordering is critical: one serial ~358GB/s engine serves
            # all queues. First on it: the 64KB of Wqk chunk 0 plus the
            # first half of xT chunk 0 — exactly what matmul #1 consumes.
            wqkt0 = work.tile([128, NCK, 256], bf16, tag="wqkt", bufs=3,
                              name="wqkt")
            xts = [
                const.tile([128, T], bf16, tag=f"xt{i}", name=f"xt{i}")
                for i in range(NCK)
            ]
            for c0, c1 in ((0, 2), (2, 6)):
                nc.gpsimd.dma_start(
                    out=wqkt0[:, c0:c1],
                    in_=xw_ap(OWQK + c0 * 256,
                              [[NCK * 256, 128], [256, c1 - c0], [1, 256]]),
                )
            nc.sync.dma_start(out=xts[0][:, 0:512],
                              in_=xw_ap(OXT, [[T, 128], [1, 512]]))
            nc.sync.dma_start(out=xts[0][:, 512:1024],
                              in_=xw_ap(OXT + 512, [[T, 128], [1, 512]]))
            for i in range(1, NCK):
                eng = nc.sync if i % 2 == 0 else nc.scalar
                eng.dma_start(out=xts[i],
                              in_=xw_ap(OXT + i * 128 * T,
                                        [[T, 128], [1, T]]))
            wv_all = const.tile([128, NCK, C], bf16, tag="wvall",
                                name="wvall")
            nc.scalar.dma_start(
                out=wv_all,
                in_=xw_ap(OWV, [[C, 128], [128 * C, NCK], [1, C]]),
            )
            wvts = [wv_all[:, i, :] for i in range(NCK)]
            # proj weights land mid-kernel: DMAs issued at pair 3 so they
            # don't compete with xT/Wv on the serial HBM engine
            wp_all = const.tile([128, NPAIR, C], bf16, tag="wpall",
                                name="wpall")
            wpts = [wp_all[:, pp, :] for pp in range(NPAIR)]
            bias_t = const.tile([128, C], bf16)

            def emit_wp_loads():
                nc.gpsimd.dma_start(
                    out=wp_all,
                    in_=xw_ap(OWP, [[C, 128], [128 * C, NPAIR], [1, C]]),
                )
                nc.gpsimd.dma_start(
                    out=bias_t,
                    in_=xw_ap(OBP, [[0, 128], [1, C]]),
                )

            U = const.tile([128, 128], bf16)
            make_upper_triangular(nc, U[:, :], val=1.0, diag=True)
            U3 = U.rearrange("p (o c) -> p o c", o=1)
            ones_f = const.tile([128, 12], f32)
            nc.vector.memset(ones_f, 1.0)
            # warm the PE p-state ramp: a tiny junk matmul at t~0.2us
            # (ones_f is the first constant ready) starts the ~3us clock
            # ramp early, so the first REAL matmuls at t~3.7us — gated on
            # the weight/x DMAs — run at full speed instead of 2-3.7x
            # slower. Costs ~20ns of PE on an otherwise idle engine.
            pewarm = ps1.tile([12, 12], f32, tag="po", bufs=4,
                              name="pewarm")
            nc.tensor.matmul(pewarm, lhsT=ones_f, rhs=ones_f,
                             start=True, stop=True)
            # selector matrices for the rowsum broadcast: for pair parity
            # e/o, ones at (r0, 0:64) and (r0+32, 64:128) pick the two
            # head rows of rs3 and replicate them across the halves
            SE = const.tile([128, 128], bf16)
            SO = const.tile([128, 128], bf16)
            for S, r0 in ((SE, 0), (SO, 64)):
                nc.vector.memset(S, 0.0)
                nc.vector.memset(S[r0:r0 + 1, 0:64], 1.0)
                nc.vector.memset(S[r0 + 32:r0 + 33, 64:128], 1.0)
            # warm the Exp activation table before the first real softmax
            warm = const.tile([1, 2], f32)
            nc.scalar.activation(out=warm, in_=ones_f[0:1, 0:2], func=AF.Exp)

            # v in token-major layout: per k-tile, 12 heads x (64 cols of v | 1)
            v_all = const.tile([128, NT, H * 65], bf16)
            v_heads = v_all.rearrange("p k (h c) -> p k h c", h=H)
            for kt in range(NT):
                nc.vector.tensor_copy(
                    out=v_heads[:, kt, :, 64:65],
                    in_=ones_f.rearrange("p (h o) -> p h o", o=1),
                )

            outTs = [
                opool.tile([128, T], bf16, tag=f"outT{p}", name=f"outT{p}")
                for p in range(NPAIR)
            ]
            rs3 = [
                opool.tile([128, T], bf16, tag=f"rs{j}", name=f"rs{j}")
                for j in range(3)
            ]
            for j in range(3):
                # selector matmul contracts over all 128 partitions of rs3;
                # zero the unwritten rows so 0*garbage can't produce NaN
                nc.gpsimd.memset(rs3[j], 0.0)

            # ---------- emit helpers (software-pipelined) ----------
            def emit_vproj(kts):
                # k-tiles interleaved in pairs: each Wv chunk arrival over
                # the serial HBM engine unlocks 2x the PE work, halving the
                # early-phase DMA-wait stalls
                with nc.named_scope("vproj"):
                    for kt in kts:
                        pv = ps1.tile([128, C], f32, tag="big2", bufs=2,
                                      name="pv")
                        for ck in range(NCK):
                            for n0, n1 in ((0, 512), (512, 768)):
                                nc.tensor.matmul(
                                    pv[:, n0:n1],
                                    lhsT=xts[ck][:, kt * 128:(kt + 1) * 128],
                                    rhs=wvts[ck][:, n0:n1],
                                    start=(ck == 0),
                                    stop=(ck == NCK - 1),
                                )
                        nc.scalar.copy(
                            out=v_heads[:, kt, :, 0:64],
                            in_=pv.rearrange("p (h c) -> p h c", h=H),
                        )

            qkts = {}

            def qkT_group(p, wqkt, qT, kTt, gi):
                dst, wo = ((qT, 0), (kTt, 128))[gi // 2]
                tch = gi % 2
                with nc.named_scope(f"qk{p}"):
                    pqk = ps1.tile([128, 512], f32, tag="po", bufs=4,
                                   name="pqk")
                    for ck in range(NCK):
                        nc.tensor.matmul(
                            pqk,
                            lhsT=wqkt[:, ck, wo:wo + 128],
                            rhs=xts[ck][:, tch * 512:(tch + 1) * 512],
                            start=(ck == 0),
                            stop=(ck == NCK - 1),
                        )
                    if not FP8_SCORES:
                        if tch == 0:
                            nc.scalar.copy(out=dst[:, 0:512], in_=pqk)
                        else:
                            nc.vector.tensor_copy(out=dst[:, 512:1024],
                                                  in_=pqk)
                        return
                    # fp8 path: host reordered the wqk columns so pqk
                    # partitions are [h0 d0-31 | h1 d0-31 | h0 d32-63 |
                    # h1 d32-63]. One fp8 copy, then two partition-shift
                    # DMAs pack the DoubleRow layout [32(d), 2(dhalf), T]
                    # per head (h0 at partitions 0-31, h1 at 32-63).
                    tmp8 = work.tile([128, 512], f8, tag="qk8", bufs=4,
                                     name="tmp8")
                    with nc.allow_low_precision(
                        reason="q/k quantized to fp8e4m3 for DoubleRow "
                               "QK^T; ~1.4e-2 of the 2e-2 budget"
                    ):
                        if tch == 0:
                            nc.scalar.copy(out=tmp8, in_=pqk)
                        else:
                            nc.vector.tensor_copy(out=tmp8, in_=pqk)
                    for i, eng in ((0, nc.gpsimd), (1, nc.sync)):
                        eng.dma_start(
                            out=dst[:, i, tch * 512:(tch + 1) * 512],
                            in_=tmp8[64 * i:64 * i + 64, :],
                        )

            def make_qkT_fillers(p, wqkt=None):
                # allocate tiles and launch the weight DMA now; the four
                # matmul groups are emitted later, dripped into exp-wait
                # bubbles of the current pair's attention. Order g0,g2,g1,g3
                # so qT[0:512] and kTt[0:512] (what the next pair's first
                # scores need) are ready earliest.
                if wqkt is None:
                    wqkt = work.tile([128, NCK, 256], bf16, tag="wqkt",
                                     bufs=3, name="wqkt")
                    nc.sync.dma_start(
                        out=wqkt,
                        in_=xw_ap(OWQK + p * 128 * NCK * 256,
                                  [[NCK * 256, 128], [256, NCK], [1, 256]]),
                    )
                if FP8_SCORES:
                    qT = work.tile([64, 2, T], f8, tag="qT", bufs=3,
                                   name="qT8")
                    kTt = work.tile([64, 2, T], f8, tag="kTt", bufs=3,
                                    name="kT8")
                else:
                    qT = work.tile([128, T], bf16, tag="qT", bufs=3,
                                   name="qT")
                    kTt = work.tile([128, T], bf16, tag="kTt", bufs=3,
                                    name="kTt")
                qkts[p] = (qT, kTt)
                return [
                    (lambda gi=gi: qkT_group(p, wqkt, qT, kTt, gi))
                    for gi in (0, 2, 1, 3)
                ]

            def make_norm_filler(p, qc):
                # deferred normalization: broadcast 1/rowsum of both heads
                # across partitions with one selector matmul, one multiply.
                # Runs as a filler inside a LATER loop slot, after the
                # reciprocal has landed, so it never stalls the PE queue.
                # (A stride-0-source partition-broadcast DMA would free
                # the PE of the matmul, but the AP lowering rejects
                # partition broadcast from SBUF.)
                qsl = slice(qc * 512, (qc + 1) * 512)
                j = p // 2
                S = SE if p % 2 == 0 else SO

                def norm():
                    with nc.named_scope(f"norm{p}_{qc}"):
                        pr = ps1.tile([128, 512], f32, tag="po", bufs=4,
                                      name="pr")
                        nc.tensor.matmul(
                            pr, lhsT=S, rhs=rs3[j][:, qsl],
                            start=True, stop=True,
                        )
                        nc.vector.tensor_mul(
                            outTs[p][:, qsl], outTs[p][:, qsl], pr
                        )
                return norm

            # ---------- phase 2: output projection ----------
            def emit_proj_tile(tt, alt=False):
                # alt=True uses two 1-bank po slots instead of the 2-bank
                # big2 slot: alternating gives the proj phase 2x PSUM
                # rotation depth, so a tile's matmuls never wait on the
                # DVE bias-add of the tile two steps back
                if alt:
                    pys = [
                        ps1.tile([128, 512], f32, tag="po", bufs=4,
                                 name="pya"),
                        ps1.tile([128, 256], f32, tag="po", bufs=4,
                                 name="pyb"),
                    ]
                else:
                    py = ps1.tile([128, C], f32, tag="big2", bufs=2,
                                  name="py")
                    pys = [py[:, 0:512], py[:, 512:768]]
                for p in range(NPAIR):
                    for (n0, n1), ph in zip(((0, 512), (512, 768)), pys):
                        nc.tensor.matmul(
                            ph[:, 0:n1 - n0] if alt else ph,
                            lhsT=outTs[p][:, tt * 128:(tt + 1) * 128],
                            rhs=wpts[p][:, n0:n1],
                            start=(p == 0),
                            stop=(p == NPAIR - 1),
                        )
                ysb = work.tile([128, C], bf16, tag="ysb", bufs=3,
                                name="ysb")
                with nc.allow_low_precision(
                    reason="y stored bf16, upcast on host; ~4e-3 of "
                           "the 2e-2 budget"
                ):
                    for (n0, n1), ph in zip(((0, 512), (512, 768)), pys):
                        nc.vector.tensor_add(
                            ysb[:, n0:n1],
                            ph[:, 0:n1 - n0] if alt else ph,
                            bias_t[:, n0:n1],
                        )
                nc.sync.dma_start(
                    out=y_d[tt * 128:(tt + 1) * 128, :], in_=ysb
                )

            # ---------- phase 1: attention as one flat pipeline ----------
            # stage = (pair, qc, k-tile); scores/exp run LOOK tiles ahead
            # of PV, across all stage boundaries
            stages = [
                (p, qc, kt)
                for p in range(NPAIR)
                for qc in (0, 1)
                for kt in range(4 * (qc + 1))
            ]
            LOOK = 6
            pending = {}
            po_pairs = {}
            fillers = []
            norm_fillers = []
            next_to_emit = [0]

            def emit_scores_until(bound):
                while next_to_emit[0] <= min(bound, len(stages) - 1):
                    emit_scores(next_to_emit[0])
                    next_to_emit[0] += 1

            def emit_scores(i):
                p, qc, kt = stages[i]
                qT, kTt = qkts[p]
                ccol = max(0, 128 * kt - 512 * qc)
                pt = ppool.tile([128, 2, 512], bf16, tag="pt", bufs=8,
                                name="pt")
                pscr = ps1.tile([128, 2, 512], f32, tag="big2", bufs=2,
                                name="pscr")
                with nc.named_scope(f"sc{p}_{qc}_{kt}"):
                    for hh in range(2):
                        if FP8_SCORES:
                            nc.tensor.matmul(
                                pscr[:, hh, ccol:512],
                                lhsT=kTt[32 * hh:32 * hh + 32, :,
                                         kt * 128:(kt + 1) * 128],
                                rhs=qT[32 * hh:32 * hh + 32, :,
                                       qc * 512 + ccol:(qc + 1) * 512],
                                start=True,
                                stop=True,
                                perf_mode=DR,
                            )
                        else:
                            nc.tensor.matmul(
                                pscr[:, hh, ccol:512],
                                lhsT=kTt[hh * 64:(hh + 1) * 64,
                                         kt * 128:(kt + 1) * 128],
                                rhs=qT[hh * 64:(hh + 1) * 64,
                                       qc * 512 + ccol:(qc + 1) * 512],
                                start=True,
                                stop=True,
                            )
                    # one Exp for both heads: the ~370ns per-instruction
                    # PSUM/SBUF access charge dominates a split
                    nc.scalar.activation(
                        out=pt[:, :, ccol:512],
                        in_=pscr[:, :, ccol:512],
                        func=AF.Exp,
                        scale=float(HS) ** -0.5,
                    )
                    if 128 * kt >= 512 * qc:
                        # diagonal tile: zero the upper triangle (DVE,
                        # all-bf16 2x mode, ~190ns)
                        nc.vector.tensor_mul(
                            pt[:, :, ccol:ccol + 128],
                            pt[:, :, ccol:ccol + 128],
                            U3.broadcast_to([128, 2, 128]),
                        )
                pending[i] = (pt, ccol)

            for f in make_qkT_fillers(0, wqkt0):
                f()

            for i, (p, qc, kt) in enumerate(stages):
                if qc == 0 and kt == 0:
                    # entering pair p
                    if p + 1 < NPAIR:
                        fillers += make_qkT_fillers(p + 1)
                    if p == 3:
                        # wp/bias needed first at pair 5 (proj0); issuing
                        # here keeps the serial DMA resource free for the
                        # startup x/wv loads AND the early fp8 remap DMAs
                        emit_wp_loads()
                if p == 0 and qc == 0 and kt == 0:
                    # deep prefill for pair 0: the v projection runs first
                    # (15us of PE work) covering the input-DMA wait and,
                    # on the fp8 path, the first qT8/kT8 remap latency
                    if FP8_SCORES:
                        emit_vproj(range(0, NT))
                        emit_scores_until(0)
                    else:
                        emit_scores_until(0)
                        emit_vproj(range(0, NT))

                emit_scores_until(i + LOOK)
                if kt == 0:
                    po_pairs[(p, qc)] = [
                        ps1.tile([65, 512], f32, tag="po", bufs=4,
                                 name=f"po{hh}")
                        for hh in range(2)
                    ]
                po_pair = po_pairs[(p, qc)]
                nkt = 4 * (qc + 1)
                pt, ccol = pending.pop(i)
                with nc.named_scope(f"pv{p}_{qc}_{kt}"):
                    for hh in range(2):
                        h = 2 * p + hh
                        nc.tensor.matmul(
                            po_pair[hh][:, ccol:512],
                            lhsT=v_all[:, kt, h * 65:(h + 1) * 65],
                            rhs=pt[:, hh, ccol:512],
                            start=(kt == 0),
                            stop=(kt == nkt - 1),
                        )
                if fillers and (FP8_SCORES or kt % 2 == 1):
                    fillers.pop(0)()
                if norm_fillers and kt == 1:
                    norm_fillers.pop(0)()
                if p == 5 and qc == 1 and kt == 4:
                    # pair 5 has no successor qkT work to absorb its tail
                    # bubbles: run proj tile 0 here instead. Its PSUM comes
                    # from two free 1-bank po-tag slots, so it does not
                    # contend with the last score tiles on the big2 slots.
                    # outT qc0-halves (all tile 0 reads) are normalized.
                    with nc.named_scope("proj0"):
                        pys = [
                            ps1.tile([128, 512], f32, tag="po", bufs=4,
                                     name=f"py0{i}")
                            for i in range(2)
                        ]
                        for (n0, n1), pyh in zip(((0, 512), (512, 768)),
                                                 pys):
                            for pp in range(NPAIR):
                                nc.tensor.matmul(
                                    pyh[:, 0:n1 - n0],
                                    lhsT=outTs[pp][:, 0:128],
                                    rhs=wpts[pp][:, n0:n1],
                                    start=(pp == 0),
                                    stop=(pp == NPAIR - 1),
                                )
                        ysb = work.tile([128, C], bf16, tag="ysb", bufs=3,
                                        name="ysb")
                        with nc.allow_low_precision(
                            reason="y stored bf16, upcast on host"
                        ):
                            for (n0, n1), pyh in zip(((0, 512), (512, 768)),
                                                     pys):
                                nc.vector.tensor_add(
                                    ysb[:, n0:n1], pyh[:, 0:n1 - n0],
                                    bias_t[:, n0:n1]
                                )
                        nc.sync.dma_start(out=y_d[0:128, :], in_=ysb)
                if p == 5 and qc == 1 and kt == 7:
                    # with LOOK=6 the last exps drained by now, so the big2
                    # slots are free: proj tile 1 also runs in the tail
                    emit_proj_tile(1)
                if kt == nkt - 1:
                    # qc finished: copies + reciprocal now (DVE), the
                    # normalize matmul+multiply deferred into later slots
                    qsl = slice(qc * 512, (qc + 1) * 512)
                    for hh in range(2):
                        h = 2 * p + hh
                        rrow = rs3[h // 4][(h % 4) * 32:(h % 4) * 32 + 1,
                                           qsl]
                        with nc.allow_low_precision(
                            reason="1/rowsum at bf16 costs ~2e-3 "
                                   "relative error; budget is 2e-2"
                        ):
                            nc.vector.reciprocal(
                                out=rrow, in_=po_pair[hh][64:65, :]
                            )
                        nc.vector.tensor_copy(
                            out=outTs[p][hh * 64:(hh + 1) * 64, qsl],
                            in_=po_pair[hh][0:64, :],
                        )
                    del po_pairs[(p, qc)]
                    if qc == 0:
                        fillers.append(make_norm_filler(p, qc))
                    else:
                        norm_fillers.append(make_norm_filler(p, qc))

            with nc.named_scope("proj"):
                # tt=0/1 already ran inside pair 5; tt=2 reads only qc=0
                # halves, so the remaining fillers (pair 5 qc=1 normalize)
                # flush behind it without stalling the PE queue
                emit_proj_tile(2)
                for f in fillers + norm_fillers:
                    f()
                fillers = []
                norm_fillers = []
                for tt in range(3, NT):
                    emit_proj_tile(tt, alt=(tt % 2 == 1))

    nc.compile()
    return nc


def get_nc():
    global _cached_nc
    if _cached_nc is None:
        _cached_nc = _build_nc()
    return _cached_nc


def _bf16(a):
    return np.asarray(a, dtype=np.float32).astype(ml_dtypes.bfloat16)


def _host_pack(inputs):
    x = np.asarray(inputs["x"], dtype=np.float32)
    Wq = np.asarray(inputs["Wq"], dtype=np.float32)
    Wk = np.asarray(inputs["Wk"], dtype=np.float32)
    Wv = np.asarray(inputs["Wv"], dtype=np.float32)
    Wproj = np.asarray(inputs["Wproj"], dtype=np.float32)
    bproj = np.asarray(inputs["bproj"], dtype=np.float32)

    Wq2 = Wq.transpose(1, 0, 2).reshape(C, C)  # [c, h*HS]
    Wk2 = Wk.transpose(1, 0, 2).reshape(C, C)
    if FP8_SCORES:
        # column order [h0 d0-31 | h1 d0-31 | h0 d32-63 | h1 d32-63] so
        # the projection PSUM lands pre-packed for the DoubleRow layout
        perm = (list(range(0, 32)) + list(range(64, 96))
                + list(range(32, 64)) + list(range(96, 128)))
    else:
        perm = list(range(128))
    wqk = np.stack(
        [
            np.concatenate(
                [
                    Wq2[:, p * 128:(p + 1) * 128][:, perm],
                    Wk2[:, p * 128:(p + 1) * 128][:, perm],
                ],
                axis=1,
            )
            for p in range(NPAIR)
        ]
    )  # [6, 768(c), 256]
    # -> [6, 128(kp), 6(ck), 256] so the on-chip tile loads contiguously
    wqk = _bf16(
        np.ascontiguousarray(
            wqk.reshape(NPAIR, NCK, 128, 256).transpose(0, 2, 1, 3)
        )
    )
    wv = _bf16(np.ascontiguousarray(Wv.transpose(1, 0, 2).reshape(C, C)))
    wp = _bf16(np.ascontiguousarray(Wproj.T.reshape(NPAIR, 128, C)))
    # merged single-input layout: xT | wqk | wv | wp | bp (all bf16)
    tail = np.concatenate(
        [wqk.ravel(), wv.ravel(), wp.ravel(), _bf16(bproj).ravel()]
    )
    assert OXT + C * T == OWQK and OBP + C == NXW
    in_maps = []
    for b in range(B):
        xw = np.empty(NXW, dtype=ml_dtypes.bfloat16)
        xw[OXT:OWQK] = _bf16(np.ascontiguousarray(x[b].T)).ravel()
        xw[OWQK:] = tail
        in_maps.append({"xw": xw})
    return in_maps


def kernel(**inputs):
    global LAST_EXEC_NS, LAST_RESULTS
    from concourse.bass_utils import run_bass_kernel_spmd

    nc = get_nc()
    in_maps = _host_pack(inputs)
    trace = bool(int(os.environ.get("KERNEL_TRACE", "0")))
    res = run_bass_kernel_spmd(
        nc, in_maps, core_ids=list(range(B)), trace=trace
    )
    LAST_EXEC_NS = res.exec_time_ns
    LAST_RESULTS = res
    y = np.stack([res.results[b]["y"] for b in range(B)])
    return y.astype(np.float32)

